# revision 19
# baseline (speedup 1.0000x reference)
"""Trainium2 Bass kernel for nn_CAM (channel-attention module).

Reference computation per sample (b=16 total):
    xf   = x.reshape(c, h*w)               # [512, 4096]
    attn = softmax(xf @ xf.T, axis=-1)     # [512, 512]
    y    = attn @ xf                       # [512, 4096]
    out  = beta * y + x

Sharding: data-parallel over batch b across 8 NeuronCores (2 samples per
core); the scalar beta is replicated (pre-broadcast to [128, 1] host-side).

Mixed-precision layout (tolerance is 2e-2; matmuls in fp8e4 DoubleRow for
2x PE throughput, I/O in bf16/fp8 to cut HBM traffic):
  - host uploads x three ways: natural bf16 [S, 128, 4, 4096]
    (partition-major swizzle) for the epilogue, natural fp8 for
    matmul2's rhs, and pre-transposed fp8 xt[s, p, j, c] = x[s, c, 128j+p]
    for matmul1 (the Gram matrix needs hw on partitions on both operands;
    transposing on the PE would cost ~30us/core of TensorE time, and
    casting on-device measured 4x slower than modeled on gpsimd).
  - matmul1 (G = xf xf^T): 16 DoubleRow MMs per c-tile (K=256 each).
  - softmax: DVE reduce_max(negate) -> ACT Exp(bias=-max) with fused
    accum_out row-sum.  The 1/rowsum * beta normalization is NOT applied
    to P; it is folded into the epilogue as a per-partition scalar.
  - P^T on the PE (16 transpose blocks), PSUM->SBUF copy casts to fp8.
  - matmul2 (y = P @ xf): 2 DoubleRow MMs per [128, 512] output chunk.
  - epilogue: one DVE scalar_tensor_tensor: out = (psum * rb_c) + x_bf16,
    rb_c = beta / rowsum_c, written as bf16 and upcast on host.
  - the two samples' phases are emitted software-pipelined
    (load0, mm1_0, load1, T_0, mm1_1, mm2_0, T_1, mm2_1) so the PE gap
    while sample s's softmax tail completes is filled by sample s+1's
    matmul1.
"""

import numpy as np
import ml_dtypes

import concourse.bass as bass
import concourse.bacc as bacc
import concourse.mybir as mybir
import concourse.tile as tile
from concourse.bass import ts
from concourse.bass_utils import run_bass_kernel_spmd
from concourse.masks import make_identity

N_CORES = 8
P = 128

F32 = mybir.dt.float32
BF16 = mybir.dt.bfloat16
FP8 = mybir.dt.float8e4

NP_BF16 = ml_dtypes.bfloat16
NP_FP8 = ml_dtypes.float8_e4m3

DR = mybir.MatmulPerfMode.DoubleRow
MM1_PERF_MODE = DR


def _mm(nc, out, lhsT, rhs, start, stop, perf_mode=None, ldw=True):
    """nc.tensor.matmul clone with control over the ldweights field.

    When several consecutive matmuls share the same stationary operand,
    walrus still emits one LDWEIGHTS per matmul (no dedupe), and the
    ~213ns weight load serializes with the ~213ns moving stream.  Passing
    ldweights=False on the repeats skips the reload and nearly doubles
    sustained DoubleRow throughput.
    """
    eng = nc.tensor
    keep_dims = {0}
    if perf_mode is not None:
        keep_dims.add(1)
    ifmap_ap = eng.lower_ap(rhs.opt(keep_dims), opt=False)
    weights_ap = eng.lower_ap(
        lhsT.opt(keep_dims), opt=False, for_matmul_weights=True
    )
    out_ap = eng.lower_ap(out)
    return eng.add_instruction(
        mybir.InstMatmult(
            name=eng.bass.get_next_instruction_name(),
            replication_resolution=0,
            replication_shift_amnt=0,
            replication_num_rows=0,
            start_tensor_calc=start,
            stop_tensor_calc=stop,
            ins=[ifmap_ap, weights_ap],
            outs=[out_ap],
            perf_mode=perf_mode,
            is_transpose=None,
            ifmap_quant_offset=None,
            weights_quant_offset=None,
            bass_skip_group_check=None,
            tile_position=(0, 0),
            tile_size=(128, 128),
            ldweights=None if ldw else False,
        )
    )


def build_program(S=2, C=512, HW=4096, n_cores=N_CORES):
    """Build the SPMD Bass program for one core holding S samples."""
    CT = C // P        # c-tiles (partition tiles of the channel dim)
    NT = HW // P       # n-blocks (contraction tiles for matmul1)
    NCHUNK = 512       # free-dim chunk for matmul2 / epilogue (one PSUM bank)
    NCH = HW // NCHUNK
    XTC = 4            # xt arrives in 4 DMA chunks so matmul1 starts early

    nc = bacc.Bacc(
        "TRN2", target_bir_lowering=False, debug=False, num_devices=n_cores
    )
    # natural x, partition-major: xb[s, p, i, n] = x[s, 128*i + p, n]
    xb_in = nc.dram_tensor("xb", [S, P, CT, HW], BF16, kind="ExternalInput").ap()
    x8_in = nc.dram_tensor("x8", [S, P, CT, HW], FP8, kind="ExternalInput").ap()
    # transposed x: xt[s, p, j, c] = x[s, c, 128*j + p]
    xt_in = nc.dram_tensor("xt", [S, P, NT, C], FP8, kind="ExternalInput").ap()
    beta_in = nc.dram_tensor("beta", [P, 1], F32, kind="ExternalInput").ap()
    out_d = nc.dram_tensor("out", [S, P, CT, HW], BF16, kind="ExternalOutput").ap()

    with tile.TileContext(nc) as tc:
        with (
            tc.tile_pool(name="consts", bufs=1) as consts,
            tc.tile_pool(name="xt", bufs=2) as xt_pool,
            tc.tile_pool(name="xb", bufs=2) as xb_pool,
            tc.tile_pool(name="x8", bufs=2) as x8_pool,
            tc.tile_pool(name="pm", bufs=2) as pm_pool,
            tc.tile_pool(name="pt", bufs=2) as pt_pool,
            tc.tile_pool(name="stats", bufs=8) as stats_pool,
            tc.tile_pool(name="outsb", bufs=3) as out_pool,
            tc.tile_pool(name="psumA", bufs=2, space="PSUM") as psumA_pool,
            tc.tile_pool(name="psumY", bufs=1, space="PSUM") as psumY_pool,
            tc.tile_pool(name="psumT", bufs=1, space="PSUM") as psumT_pool,
        ):
            beta_bc = consts.tile([P, 1], F32)
            nc.sync.dma_start(beta_bc[:], beta_in)
            ident = consts.tile([P, P], BF16)
            make_identity(nc, ident[:])

            # ~3.5us of dummy matmuls during the initial DMA fill: HAM
            # un-throttles the PE clock (4/8 -> 8/8) after one busy window,
            # so the real matmul1 starts warm.
            warm = psumA_pool.tile([P, C], F32, tag="psumA", name="warm")
            for _ in range(32):
                nc.tensor.matmul(
                    warm[:, 0:P], lhsT=ident[:], rhs=ident[:], start=True, stop=True
                )

            # per-sample state threaded between phases
            st = [dict() for _ in range(S)]

            def load_phase(s):
                xt_t = xt_pool.tile([P, NT, C], FP8, tag="xt")
                for c in range(XTC):
                    nc.sync.dma_start(
                        xt_t[:, ts(c, NT // XTC), :],
                        xt_in[s, :, ts(c, NT // XTC), :],
                    )
                xb_t = xb_pool.tile([P, CT, HW], BF16, tag="xb")
                x8_t = x8_pool.tile([P, CT, HW], FP8, tag="x8")
                for i in range(CT):
                    nc.sync.dma_start(x8_t[:, i, :], x8_in[s, :, i, :])
                for i in range(CT):
                    nc.sync.dma_start(xb_t[:, i, :], xb_in[s, :, i, :])
                st[s].update(xt=xt_t, xb=xb_t, x8=x8_t)

            def mm1_phase(s):
                xt_t = st[s]["xt"]
                pm = pm_pool.tile([P, CT, C], BF16, tag="pm")
                rb = stats_pool.tile([P, CT], F32, tag="rb")
                for i in range(CT):
                    pa = psumA_pool.tile([P, C], F32, tag="psumA")
                    for t in range(NT // 2):
                        # two N=256 half-row streams off one weight load
                        for h in range(2):
                            _mm(
                                nc,
                                pa[:, ts(h, C // 2)],
                                lhsT=xt_t[:, 2 * t : 2 * t + 2, ts(i, P)],
                                rhs=xt_t[:, 2 * t : 2 * t + 2, ts(h, C // 2)],
                                start=(t == 0),
                                stop=(t == NT // 2 - 1),
                                perf_mode=MM1_PERF_MODE,
                                ldw=(h == 0),
                            )
                    negm = stats_pool.tile([P, 1], F32, tag="negm")
                    nc.vector.reduce_max(
                        negm[:], pa[:], axis=mybir.AxisListType.X, negate=True
                    )
                    ssum = stats_pool.tile([P, 1], F32, tag="ssum")
                    nc.scalar.activation(
                        pm[:, i, :],
                        pa[:],
                        mybir.ActivationFunctionType.Exp,
                        bias=negm[:],
                        scale=1.0,
                        accum_out=ssum[:],
                    )
                    # rb = beta / rowsum; applied in the epilogue
                    rinv = stats_pool.tile([P, 1], F32, tag="rinv")
                    nc.vector.reciprocal(rinv[:], ssum[:])
                    nc.vector.tensor_scalar_mul(
                        rb[:, i : i + 1], rinv[:], beta_bc[:, 0:1]
                    )
                st[s].update(pm=pm, rb=rb)

            def t_phase(s):
                # P^T on PE: PT[p, k, c] = exp(A - m)[c, 128k+p]
                pm = st[s]["pm"]
                PT = pt_pool.tile([P, CT, C], FP8, tag="PT")
                tp = psumT_pool.tile([P, CT, C], BF16, tag="psumT")
                # i-major: the 12 transposes not gated on exp(i=3) run first
                for i in range(CT):
                    for k in range(CT):
                        nc.tensor.transpose(
                            tp[:, k, ts(i, P)], pm[:, i, ts(k, P)], ident[:]
                        )
                for k in range(CT):
                    nc.scalar.copy(PT[:, k, :], tp[:, k, :])
                st[s].update(PT=PT)

            def mm2_phase(s):
                xb_t, x8_t, PT, rb = (
                    st[s]["xb"], st[s]["x8"], st[s]["PT"], st[s]["rb"]
                )
                # t-outer / n-inner over 4-chunk groups: the stationary weight
                # PT[:, pair, i] is reused across 4 moving streams, amortizing
                # LDWEIGHTS.  Each group uses two 2-bank PSUM tiles; tile 0's
                # epilogue runs as one DVE scalar_tensor_tensor, tile 1's is
                # split ACT scaled-copy + DVE bf16 add (2x DVE rate) so the
                # PSUM drain doesn't gate the matmul stream.
                for i in range(CT):
                    ot = out_pool.tile([P, HW], BF16, tag="outsb")
                    for g in range(NCH // 4):
                        pys = [
                            psumY_pool.tile(
                                [P, 2, NCHUNK], F32, tag=f"psumY{q}", name=f"py{q}"
                            )
                            for q in range(2)
                        ]
                        for t in range(CT // 2):
                            for q in range(2):
                                for j in range(2):
                                    n = g * 4 + q * 2 + j
                                    _mm(
                                        nc,
                                        pys[q][:, j, :],
                                        lhsT=PT[:, 2 * t : 2 * t + 2, ts(i, P)],
                                        rhs=x8_t[:, 2 * t : 2 * t + 2, ts(n, NCHUNK)],
                                        start=(t == 0),
                                        stop=(t == CT // 2 - 1),
                                        perf_mode=DR,
                                        ldw=(q == 0 and j == 0),
                                    )
                        # out = (y * beta/rowsum) + x   over [P, 1024] halves
                        for q in range(2):
                            nc.vector.scalar_tensor_tensor(
                                out=ot[:, ts(2 * g + q, 2 * NCHUNK)],
                                in0=pys[q][:],
                                scalar=rb[:, i : i + 1],
                                in1=xb_t[:, i, ts(2 * g + q, 2 * NCHUNK)],
                                op0=mybir.AluOpType.mult,
                                op1=mybir.AluOpType.add,
                            )
                    for h in range(2):
                        nc.sync.dma_start(
                            out_d[s, :, i, ts(h, HW // 2)], ot[:, ts(h, HW // 2)]
                        )

            # software-pipelined emission over the S=2 samples
            load_phase(0)
            mm1_phase(0)
            load_phase(1)
            t_phase(0)
            mm1_phase(1)
            mm2_phase(0)
            t_phase(1)
            mm2_phase(1)

    nc.compile()
    return nc


_PROGRAM_CACHE = {}


def _get_program(S, C, HW, n_cores):
    key = (S, C, HW, n_cores)
    if key not in _PROGRAM_CACHE:
        _PROGRAM_CACHE[key] = build_program(S, C, HW, n_cores)
    return _PROGRAM_CACHE[key]


def make_in_maps(x: np.ndarray, beta: np.ndarray):
    """Host-side prep: shard over batch, swizzle + downcast both layouts."""
    b, c, h, w = x.shape
    hw = h * w
    S = b // N_CORES
    CT = c // P
    NT = hw // P

    xf = np.asarray(x, dtype=np.float32).reshape(b, c, hw)
    # natural, partition-major: [b, P, CT, HW]
    xn = np.ascontiguousarray(xf.reshape(b, CT, P, hw).transpose(0, 2, 1, 3))
    xb = xn.astype(NP_BF16)
    x8 = xn.astype(NP_FP8)
    # transposed: xt[s, p, j, c] = x[s, c, 128j+p] -> [b, P, NT, C]
    xt = np.ascontiguousarray(
        xf.reshape(b, c, NT, P).transpose(0, 3, 2, 1)
    ).astype(NP_FP8)
    beta_bc = np.ascontiguousarray(
        np.broadcast_to(np.asarray(beta, dtype=np.float32).reshape(1, 1), (P, 1))
    )
    return [
        {
            "xb": xb[core * S : (core + 1) * S],
            "x8": x8[core * S : (core + 1) * S],
            "xt": xt[core * S : (core + 1) * S],
            "beta": beta_bc,
        }
        for core in range(N_CORES)
    ]


def kernel(x: np.ndarray, beta: np.ndarray) -> np.ndarray:
    b, c, h, w = x.shape
    assert (b, c, h, w) == (16, 512, 64, 64), f"unexpected shape {x.shape}"
    hw = h * w
    S = b // N_CORES
    CT = c // P

    nc = _get_program(S, c, hw, N_CORES)
    in_maps = make_in_maps(x, beta)
    res = run_bass_kernel_spmd(nc, in_maps, list(range(N_CORES)))

    out = np.empty((b, P, CT, hw), dtype=NP_BF16)
    for core in range(N_CORES):
        out[core * S : (core + 1) * S] = res.results[core]["out"]
    # [b, P, CT, HW] -> [b, C, HW] fp32
    out = out.transpose(0, 2, 1, 3).astype(np.float32).reshape(b, c, hw)
    return out.reshape(b, c, h, w)


# revision 22
# speedup vs baseline: 1.0055x; 1.0055x over previous
"""Trainium2 Bass kernel for nn_CAM (channel-attention module).

Reference computation per sample (b=16 total):
    xf   = x.reshape(c, h*w)               # [512, 4096]
    attn = softmax(xf @ xf.T, axis=-1)     # [512, 512]
    y    = attn @ xf                       # [512, 4096]
    out  = beta * y + x

Sharding: data-parallel over batch b across 8 NeuronCores (2 samples per
core); the scalar beta is replicated (pre-broadcast to [128, 1] host-side).

Mixed-precision layout (tolerance is 2e-2; matmuls in fp8e4 DoubleRow for
2x PE throughput, I/O in bf16/fp8 to cut HBM traffic):
  - host uploads x three ways: natural bf16 [S, 128, 4, 4096]
    (partition-major swizzle) for the epilogue, natural fp8 for
    matmul2's rhs, and pre-transposed fp8 xt[s, p, j, c] = x[s, c, 128j+p]
    for matmul1 (the Gram matrix needs hw on partitions on both operands;
    transposing on the PE would cost ~30us/core of TensorE time, and
    casting on-device measured 4x slower than modeled on gpsimd).
  - matmul1 (G = xf xf^T): 16 DoubleRow MMs per c-tile (K=256 each).
  - softmax: DVE reduce_max(negate) -> ACT Exp(bias=-max) with fused
    accum_out row-sum.  The 1/rowsum * beta normalization is NOT applied
    to P; it is folded into the epilogue as a per-partition scalar.
  - P^T on the PE (16 transpose blocks), PSUM->SBUF copy casts to fp8.
  - matmul2 (y = P @ xf): 2 DoubleRow MMs per [128, 512] output chunk.
  - epilogue: one DVE scalar_tensor_tensor: out = (psum * rb_c) + x_bf16,
    rb_c = beta / rowsum_c, written as bf16 and upcast on host.
  - the two samples' phases are emitted software-pipelined
    (load0, mm1_0, load1, T_0, mm1_1, mm2_0, T_1, mm2_1) so the PE gap
    while sample s's softmax tail completes is filled by sample s+1's
    matmul1.
"""

import numpy as np
import ml_dtypes

import concourse.bass as bass
import concourse.bacc as bacc
import concourse.mybir as mybir
import concourse.tile as tile
from concourse.bass import ts
from concourse.bass_utils import run_bass_kernel_spmd
from concourse.masks import make_identity

N_CORES = 8
P = 128

F32 = mybir.dt.float32
BF16 = mybir.dt.bfloat16
FP8 = mybir.dt.float8e4

NP_BF16 = ml_dtypes.bfloat16
NP_FP8 = ml_dtypes.float8_e4m3

DR = mybir.MatmulPerfMode.DoubleRow
MM1_PERF_MODE = DR


def _mm(nc, out, lhsT, rhs, start, stop, perf_mode=None, ldw=True):
    """nc.tensor.matmul clone with control over the ldweights field.

    When several consecutive matmuls share the same stationary operand,
    walrus still emits one LDWEIGHTS per matmul (no dedupe), and the
    ~213ns weight load serializes with the ~213ns moving stream.  Passing
    ldweights=False on the repeats skips the reload and nearly doubles
    sustained DoubleRow throughput.
    """
    eng = nc.tensor
    keep_dims = {0}
    if perf_mode is not None:
        keep_dims.add(1)
    ifmap_ap = eng.lower_ap(rhs.opt(keep_dims), opt=False)
    weights_ap = eng.lower_ap(
        lhsT.opt(keep_dims), opt=False, for_matmul_weights=True
    )
    out_ap = eng.lower_ap(out)
    return eng.add_instruction(
        mybir.InstMatmult(
            name=eng.bass.get_next_instruction_name(),
            replication_resolution=0,
            replication_shift_amnt=0,
            replication_num_rows=0,
            start_tensor_calc=start,
            stop_tensor_calc=stop,
            ins=[ifmap_ap, weights_ap],
            outs=[out_ap],
            perf_mode=perf_mode,
            is_transpose=None,
            ifmap_quant_offset=None,
            weights_quant_offset=None,
            bass_skip_group_check=None,
            tile_position=(0, 0),
            tile_size=(128, 128),
            ldweights=None if ldw else False,
        )
    )


def build_program(S=2, C=512, HW=4096, n_cores=N_CORES):
    """Build the SPMD Bass program for one core holding S samples."""
    CT = C // P        # c-tiles (partition tiles of the channel dim)
    NT = HW // P       # n-blocks (contraction tiles for matmul1)
    NCHUNK = 512       # free-dim chunk for matmul2 / epilogue (one PSUM bank)
    NCH = HW // NCHUNK
    XTC = 4            # xt arrives in 4 DMA chunks so matmul1 starts early

    nc = bacc.Bacc(
        "TRN2", target_bir_lowering=False, debug=False, num_devices=n_cores
    )
    # natural x, partition-major: xb[s, p, i, n] = x[s, 128*i + p, n]
    xb_in = nc.dram_tensor("xb", [S, P, CT, HW], BF16, kind="ExternalInput").ap()
    x8_in = nc.dram_tensor("x8", [S, P, CT, HW], FP8, kind="ExternalInput").ap()
    # transposed x: xt[s, p, j, c] = x[s, c, 128*j + p]
    xt_in = nc.dram_tensor("xt", [S, P, NT, C], FP8, kind="ExternalInput").ap()
    beta_in = nc.dram_tensor("beta", [P, 1], F32, kind="ExternalInput").ap()
    out_d = nc.dram_tensor("out", [S, P, CT, HW], BF16, kind="ExternalOutput").ap()

    with tile.TileContext(nc) as tc:
        with (
            tc.tile_pool(name="consts", bufs=1) as consts,
            tc.tile_pool(name="xt", bufs=2) as xt_pool,
            tc.tile_pool(name="xb", bufs=2) as xb_pool,
            tc.tile_pool(name="x8", bufs=2) as x8_pool,
            tc.tile_pool(name="pm", bufs=2) as pm_pool,
            tc.tile_pool(name="pt", bufs=2) as pt_pool,
            tc.tile_pool(name="stats", bufs=8) as stats_pool,
            tc.tile_pool(name="sc", bufs=3) as sc_pool,
            tc.tile_pool(name="outsb", bufs=3) as out_pool,
            tc.tile_pool(name="psumA", bufs=2, space="PSUM") as psumA_pool,
            tc.tile_pool(name="psumY", bufs=1, space="PSUM") as psumY_pool,
            tc.tile_pool(name="psumT", bufs=1, space="PSUM") as psumT_pool,
        ):
            beta_bc = consts.tile([P, 1], F32)
            nc.sync.dma_start(beta_bc[:], beta_in)
            ident = consts.tile([P, P], BF16)
            make_identity(nc, ident[:])

            # ~3.5us of dummy matmuls during the initial DMA fill: HAM
            # un-throttles the PE clock (4/8 -> 8/8) after one busy window,
            # so the real matmul1 starts warm.
            warm = psumA_pool.tile([P, C], F32, tag="psumA", name="warm")
            for _ in range(32):
                nc.tensor.matmul(
                    warm[:, 0:P], lhsT=ident[:], rhs=ident[:], start=True, stop=True
                )

            # per-sample state threaded between phases
            st = [dict() for _ in range(S)]

            def load_phase(s):
                xt_t = xt_pool.tile([P, NT, C], FP8, tag="xt")
                for c in range(XTC):
                    nc.sync.dma_start(
                        xt_t[:, ts(c, NT // XTC), :],
                        xt_in[s, :, ts(c, NT // XTC), :],
                    )
                xb_t = xb_pool.tile([P, CT, HW], BF16, tag="xb")
                x8_t = x8_pool.tile([P, CT, HW], FP8, tag="x8")
                for i in range(CT):
                    nc.sync.dma_start(x8_t[:, i, :], x8_in[s, :, i, :])
                for i in range(CT):
                    nc.sync.dma_start(xb_t[:, i, :], xb_in[s, :, i, :])
                st[s].update(xt=xt_t, xb=xb_t, x8=x8_t)

            def mm1_phase(s):
                xt_t = st[s]["xt"]
                pm = pm_pool.tile([P, CT, C], BF16, tag="pm")
                rb = stats_pool.tile([P, CT], F32, tag="rb")
                for i in range(CT):
                    pa = psumA_pool.tile([P, C], F32, tag="psumA")
                    for t in range(NT // 2):
                        nc.tensor.matmul(
                            pa[:],
                            lhsT=xt_t[:, 2 * t : 2 * t + 2, ts(i, P)],
                            rhs=xt_t[:, 2 * t : 2 * t + 2, :],
                            start=(t == 0),
                            stop=(t == NT // 2 - 1),
                            perf_mode=MM1_PERF_MODE,
                        )
                    negm = stats_pool.tile([P, 1], F32, tag="negm")
                    nc.vector.reduce_max(
                        negm[:], pa[:], axis=mybir.AxisListType.X, negate=True
                    )
                    ssum = stats_pool.tile([P, 1], F32, tag="ssum")
                    nc.scalar.activation(
                        pm[:, i, :],
                        pa[:],
                        mybir.ActivationFunctionType.Exp,
                        bias=negm[:],
                        scale=1.0,
                        accum_out=ssum[:],
                    )
                    # rb = beta / rowsum; applied in the epilogue
                    rinv = stats_pool.tile([P, 1], F32, tag="rinv")
                    nc.vector.reciprocal(rinv[:], ssum[:])
                    nc.vector.tensor_scalar_mul(
                        rb[:, i : i + 1], rinv[:], beta_bc[:, 0:1]
                    )
                st[s].update(pm=pm, rb=rb)

            def t_phase(s):
                # P^T on PE: PT[p, k, c] = exp(A - m)[c, 128k+p]
                pm = st[s]["pm"]
                PT = pt_pool.tile([P, CT, C], FP8, tag="PT")
                tp = psumT_pool.tile([P, CT, C], BF16, tag="psumT")
                # i-major: the 12 transposes not gated on exp(i=3) run first
                for i in range(CT):
                    for k in range(CT):
                        nc.tensor.transpose(
                            tp[:, k, ts(i, P)], pm[:, i, ts(k, P)], ident[:]
                        )
                for k in range(CT):
                    nc.scalar.copy(PT[:, k, :], tp[:, k, :])
                st[s].update(PT=PT)

            def mm2_phase(s):
                xb_t, x8_t, PT, rb = (
                    st[s]["xb"], st[s]["x8"], st[s]["PT"], st[s]["rb"]
                )
                # t-outer / n-inner over 4-chunk groups: the stationary weight
                # PT[:, pair, i] is reused across 4 moving streams, amortizing
                # LDWEIGHTS.  Each group uses two 2-bank PSUM tiles; tile 0's
                # epilogue runs as one DVE scalar_tensor_tensor, tile 1's is
                # split ACT scaled-copy + DVE bf16 add (2x DVE rate) so the
                # PSUM drain doesn't gate the matmul stream.
                for i in range(CT):
                    ot = out_pool.tile([P, HW], BF16, tag="outsb")
                    for g in range(NCH // 4):
                        pys = [
                            psumY_pool.tile(
                                [P, 2, NCHUNK], F32, tag=f"psumY{q}", name=f"py{q}"
                            )
                            for q in range(2)
                        ]
                        for t in range(CT // 2):
                            for q in range(2):
                                for j in range(2):
                                    n = g * 4 + q * 2 + j
                                    nc.tensor.matmul(
                                        pys[q][:, j, :],
                                        lhsT=PT[:, 2 * t : 2 * t + 2, ts(i, P)],
                                        rhs=x8_t[:, 2 * t : 2 * t + 2, ts(n, NCHUNK)],
                                        start=(t == 0),
                                        stop=(t == CT // 2 - 1),
                                        perf_mode=DR,
                                    )
                        # out = (y * beta/rowsum) + x   over [P, 1024] halves.
                        # ~2/3 of the tiles drain via ACT scaled-copy + 2x-rate
                        # DVE bf16 add; the rest via one DVE STT.  This spreads
                        # the fp32 PSUM reads (the mm2 pace-setter) over both
                        # engines.
                        for q in range(2):
                            idx = (i * 2 + g) * 2 + q
                            if idx % 3 == 0:
                                nc.vector.scalar_tensor_tensor(
                                    out=ot[:, ts(2 * g + q, 2 * NCHUNK)],
                                    in0=pys[q][:],
                                    scalar=rb[:, i : i + 1],
                                    in1=xb_t[:, i, ts(2 * g + q, 2 * NCHUNK)],
                                    op0=mybir.AluOpType.mult,
                                    op1=mybir.AluOpType.add,
                                )
                            else:
                                sc = sc_pool.tile(
                                    [P, 2 * NCHUNK], BF16, tag="sc"
                                )
                                nc.scalar.mul(sc[:], pys[q][:], rb[:, i : i + 1])
                                nc.vector.tensor_add(
                                    out=ot[:, ts(2 * g + q, 2 * NCHUNK)],
                                    in0=sc[:],
                                    in1=xb_t[:, i, ts(2 * g + q, 2 * NCHUNK)],
                                )
                    for h in range(2):
                        nc.sync.dma_start(
                            out_d[s, :, i, ts(h, HW // 2)], ot[:, ts(h, HW // 2)]
                        )

            # software-pipelined emission over the S=2 samples
            load_phase(0)
            mm1_phase(0)
            load_phase(1)
            t_phase(0)
            mm1_phase(1)
            mm2_phase(0)
            t_phase(1)
            mm2_phase(1)

    nc.compile()
    return nc


_PROGRAM_CACHE = {}


def _get_program(S, C, HW, n_cores):
    key = (S, C, HW, n_cores)
    if key not in _PROGRAM_CACHE:
        _PROGRAM_CACHE[key] = build_program(S, C, HW, n_cores)
    return _PROGRAM_CACHE[key]


def make_in_maps(x: np.ndarray, beta: np.ndarray):
    """Host-side prep: shard over batch, swizzle + downcast both layouts."""
    b, c, h, w = x.shape
    hw = h * w
    S = b // N_CORES
    CT = c // P
    NT = hw // P

    xf = np.asarray(x, dtype=np.float32).reshape(b, c, hw)
    # natural, partition-major: [b, P, CT, HW]
    xn = np.ascontiguousarray(xf.reshape(b, CT, P, hw).transpose(0, 2, 1, 3))
    xb = xn.astype(NP_BF16)
    x8 = xn.astype(NP_FP8)
    # transposed: xt[s, p, j, c] = x[s, c, 128j+p] -> [b, P, NT, C]
    xt = np.ascontiguousarray(
        xf.reshape(b, c, NT, P).transpose(0, 3, 2, 1)
    ).astype(NP_FP8)
    beta_bc = np.ascontiguousarray(
        np.broadcast_to(np.asarray(beta, dtype=np.float32).reshape(1, 1), (P, 1))
    )
    return [
        {
            "xb": xb[core * S : (core + 1) * S],
            "x8": x8[core * S : (core + 1) * S],
            "xt": xt[core * S : (core + 1) * S],
            "beta": beta_bc,
        }
        for core in range(N_CORES)
    ]


def kernel(x: np.ndarray, beta: np.ndarray) -> np.ndarray:
    b, c, h, w = x.shape
    assert (b, c, h, w) == (16, 512, 64, 64), f"unexpected shape {x.shape}"
    hw = h * w
    S = b // N_CORES
    CT = c // P

    nc = _get_program(S, c, hw, N_CORES)
    in_maps = make_in_maps(x, beta)
    res = run_bass_kernel_spmd(nc, in_maps, list(range(N_CORES)))

    out = np.empty((b, P, CT, hw), dtype=NP_BF16)
    for core in range(N_CORES):
        out[core * S : (core + 1) * S] = res.results[core]["out"]
    # [b, P, CT, HW] -> [b, C, HW] fp32
    out = out.transpose(0, 2, 1, 3).astype(np.float32).reshape(b, c, hw)
    return out.reshape(b, c, h, w)


# revision 26
# speedup vs baseline: 1.0199x; 1.0143x over previous
"""Trainium2 Bass kernel for nn_CAM (channel-attention module).

Reference computation per sample (b=16 total):
    xf   = x.reshape(c, h*w)               # [512, 4096]
    attn = softmax(xf @ xf.T, axis=-1)     # [512, 512]
    y    = attn @ xf                       # [512, 4096]
    out  = beta * y + x

Sharding: data-parallel over batch b across 8 NeuronCores (2 samples per
core); the scalar beta is replicated (pre-broadcast to [128, 1] host-side).

Mixed-precision layout (tolerance is 2e-2; matmuls in fp8e4 DoubleRow for
2x PE throughput, I/O in bf16/fp8 to cut HBM traffic):
  - host uploads x three ways: natural bf16 [S, 128, 4, 4096]
    (partition-major swizzle) for the epilogue, natural fp8 for
    matmul2's rhs, and pre-transposed fp8 xt[s, p, j, c] = x[s, c, 128j+p]
    for matmul1 (the Gram matrix needs hw on partitions on both operands;
    transposing on the PE would cost ~30us/core of TensorE time, and
    casting on-device measured 4x slower than modeled on gpsimd).
  - matmul1 (G = xf xf^T): 16 DoubleRow MMs per c-tile (K=256 each).
  - softmax: DVE reduce_max(negate) -> ACT Exp(bias=-max) with fused
    accum_out row-sum.  The 1/rowsum * beta normalization is NOT applied
    to P; it is folded into the epilogue as a per-partition scalar.
  - P^T on the PE (16 transpose blocks), PSUM->SBUF copy casts to fp8.
  - matmul2 (y = P @ xf): 2 DoubleRow MMs per [128, 512] output chunk.
  - epilogue: one DVE scalar_tensor_tensor: out = (psum * rb_c) + x_bf16,
    rb_c = beta / rowsum_c, written as bf16 and upcast on host.
  - the two samples' phases are emitted software-pipelined
    (load0, mm1_0, load1, T_0, mm1_1, mm2_0, T_1, mm2_1) so the PE gap
    while sample s's softmax tail completes is filled by sample s+1's
    matmul1.
"""

import numpy as np
import ml_dtypes

import concourse.bass as bass
import concourse.bacc as bacc
import concourse.mybir as mybir
import concourse.tile as tile
from concourse.bass import ts
from concourse.bass_utils import run_bass_kernel_spmd
from concourse.masks import make_identity

N_CORES = 8
P = 128

F32 = mybir.dt.float32
BF16 = mybir.dt.bfloat16
FP8 = mybir.dt.float8e4

NP_BF16 = ml_dtypes.bfloat16
NP_FP8 = ml_dtypes.float8_e4m3

DR = mybir.MatmulPerfMode.DoubleRow
MM1_PERF_MODE = DR


def _mm(nc, out, lhsT, rhs, start, stop, perf_mode=None, ldw=True):
    """nc.tensor.matmul clone with control over the ldweights field.

    When several consecutive matmuls share the same stationary operand,
    walrus still emits one LDWEIGHTS per matmul (no dedupe), and the
    ~213ns weight load serializes with the ~213ns moving stream.  Passing
    ldweights=False on the repeats skips the reload and nearly doubles
    sustained DoubleRow throughput.
    """
    eng = nc.tensor
    keep_dims = {0}
    if perf_mode is not None:
        keep_dims.add(1)
    ifmap_ap = eng.lower_ap(rhs.opt(keep_dims), opt=False)
    weights_ap = eng.lower_ap(
        lhsT.opt(keep_dims), opt=False, for_matmul_weights=True
    )
    out_ap = eng.lower_ap(out)
    return eng.add_instruction(
        mybir.InstMatmult(
            name=eng.bass.get_next_instruction_name(),
            replication_resolution=0,
            replication_shift_amnt=0,
            replication_num_rows=0,
            start_tensor_calc=start,
            stop_tensor_calc=stop,
            ins=[ifmap_ap, weights_ap],
            outs=[out_ap],
            perf_mode=perf_mode,
            is_transpose=None,
            ifmap_quant_offset=None,
            weights_quant_offset=None,
            bass_skip_group_check=None,
            tile_position=(0, 0),
            tile_size=(128, 128),
            ldweights=None if ldw else False,
        )
    )


def build_program(S=2, C=512, HW=4096, n_cores=N_CORES):
    """Build the SPMD Bass program for one core holding S samples."""
    CT = C // P        # c-tiles (partition tiles of the channel dim)
    NT = HW // P       # n-blocks (contraction tiles for matmul1)
    NCHUNK = 512       # free-dim chunk for matmul2 / epilogue (one PSUM bank)
    NCH = HW // NCHUNK
    XTC = 8            # xt arrives in 8 DMA chunks so matmul1 starts early

    nc = bacc.Bacc(
        "TRN2", target_bir_lowering=False, debug=False, num_devices=n_cores
    )
    # natural x, partition-major: xb[s, p, i, n] = x[s, 128*i + p, n]
    xb_in = nc.dram_tensor("xb", [S, P, CT, HW], BF16, kind="ExternalInput").ap()
    x8_in = nc.dram_tensor("x8", [S, P, CT, HW], FP8, kind="ExternalInput").ap()
    # transposed x: xt[s, p, j, c] = x[s, c, 128*j + p]
    xt_in = nc.dram_tensor("xt", [S, P, NT, C], FP8, kind="ExternalInput").ap()
    beta_in = nc.dram_tensor("beta", [P, 1], F32, kind="ExternalInput").ap()
    out_d = nc.dram_tensor("out", [S, P, CT, HW], BF16, kind="ExternalOutput").ap()

    with tile.TileContext(nc) as tc:
        with (
            tc.tile_pool(name="consts", bufs=1) as consts,
            tc.tile_pool(name="xt", bufs=2) as xt_pool,
            tc.tile_pool(name="xb", bufs=2) as xb_pool,
            tc.tile_pool(name="x8", bufs=2) as x8_pool,
            tc.tile_pool(name="pm", bufs=2) as pm_pool,
            tc.tile_pool(name="pt", bufs=2) as pt_pool,
            tc.tile_pool(name="stats", bufs=8) as stats_pool,
            tc.tile_pool(name="sc", bufs=3) as sc_pool,
            tc.tile_pool(name="outsb", bufs=3) as out_pool,
            tc.tile_pool(name="psumA", bufs=2, space="PSUM") as psumA_pool,
            tc.tile_pool(name="psumY", bufs=1, space="PSUM") as psumY_pool,
            tc.tile_pool(name="psumT", bufs=1, space="PSUM") as psumT_pool,
        ):
            beta_bc = consts.tile([P, 1], F32)
            nc.sync.dma_start(beta_bc[:], beta_in)
            ident = consts.tile([P, P], BF16)
            make_identity(nc, ident[:])

            # per-sample state threaded between phases
            st = [dict() for _ in range(S)]

            def load_phase(s):
                xt_t = xt_pool.tile([P, NT, C], FP8, tag="xt")
                for c in range(XTC):
                    nc.sync.dma_start(
                        xt_t[:, ts(c, NT // XTC), :],
                        xt_in[s, :, ts(c, NT // XTC), :],
                    )
                xb_t = xb_pool.tile([P, CT, HW], BF16, tag="xb")
                x8_t = x8_pool.tile([P, CT, HW], FP8, tag="x8")
                for i in range(CT):
                    nc.sync.dma_start(x8_t[:, i, :], x8_in[s, :, i, :])
                for i in range(CT):
                    nc.sync.dma_start(xb_t[:, i, :], xb_in[s, :, i, :])
                st[s].update(xt=xt_t, xb=xb_t, x8=x8_t)

            def mm1_phase(s):
                xt_t = st[s]["xt"]
                pm = pm_pool.tile([P, CT, C], BF16, tag="pm")
                rb = stats_pool.tile([P, CT], F32, tag="rb")
                for i in range(CT):
                    pa = psumA_pool.tile([P, C], F32, tag="psumA")
                    for t in range(NT // 2):
                        nc.tensor.matmul(
                            pa[:],
                            lhsT=xt_t[:, 2 * t : 2 * t + 2, ts(i, P)],
                            rhs=xt_t[:, 2 * t : 2 * t + 2, :],
                            start=(t == 0),
                            stop=(t == NT // 2 - 1),
                            perf_mode=MM1_PERF_MODE,
                        )
                    negm = stats_pool.tile([P, 1], F32, tag="negm")
                    nc.vector.reduce_max(
                        negm[:], pa[:], axis=mybir.AxisListType.X, negate=True
                    )
                    ssum = stats_pool.tile([P, 1], F32, tag="ssum")
                    nc.scalar.activation(
                        pm[:, i, :],
                        pa[:],
                        mybir.ActivationFunctionType.Exp,
                        bias=negm[:],
                        scale=1.0,
                        accum_out=ssum[:],
                    )
                    # rb = beta / rowsum; applied in the epilogue
                    rinv = stats_pool.tile([P, 1], F32, tag="rinv")
                    nc.vector.reciprocal(rinv[:], ssum[:])
                    nc.vector.tensor_scalar_mul(
                        rb[:, i : i + 1], rinv[:], beta_bc[:, 0:1]
                    )
                st[s].update(pm=pm, rb=rb)

            def t_phase(s):
                # P^T on PE: PT[p, k, c] = exp(A - m)[c, 128k+p]
                pm = st[s]["pm"]
                PT = pt_pool.tile([P, CT, C], FP8, tag="PT")
                tp = psumT_pool.tile([P, CT, C], BF16, tag="psumT")
                # i-major: the 12 transposes not gated on exp(i=3) run first
                for i in range(CT):
                    for k in range(CT):
                        nc.tensor.transpose(
                            tp[:, k, ts(i, P)], pm[:, i, ts(k, P)], ident[:]
                        )
                for k in range(CT):
                    nc.scalar.copy(PT[:, k, :], tp[:, k, :])
                st[s].update(PT=PT)

            def mm2_phase(s):
                xb_t, x8_t, PT, rb = (
                    st[s]["xb"], st[s]["x8"], st[s]["PT"], st[s]["rb"]
                )
                # t-outer / n-inner over 4-chunk groups: the stationary weight
                # PT[:, pair, i] is reused across 4 moving streams, amortizing
                # LDWEIGHTS.  Each group uses two 2-bank PSUM tiles; tile 0's
                # epilogue runs as one DVE scalar_tensor_tensor, tile 1's is
                # split ACT scaled-copy + DVE bf16 add (2x DVE rate) so the
                # PSUM drain doesn't gate the matmul stream.
                for i in range(CT):
                    ot = out_pool.tile([P, HW], BF16, tag="outsb")
                    for g in range(NCH // 4):
                        pys = [
                            psumY_pool.tile(
                                [P, 2, NCHUNK], F32, tag=f"psumY{q}", name=f"py{q}"
                            )
                            for q in range(2)
                        ]
                        for t in range(CT // 2):
                            for q in range(2):
                                for j in range(2):
                                    n = g * 4 + q * 2 + j
                                    nc.tensor.matmul(
                                        pys[q][:, j, :],
                                        lhsT=PT[:, 2 * t : 2 * t + 2, ts(i, P)],
                                        rhs=x8_t[:, 2 * t : 2 * t + 2, ts(n, NCHUNK)],
                                        start=(t == 0),
                                        stop=(t == CT // 2 - 1),
                                        perf_mode=DR,
                                    )
                        # out = (y * beta/rowsum) + x   over [P, 1024] halves
                        # (DVE only: ACT reads PSUM at half DVE's rate, Pool
                        # at a quarter -- measured, both lose)
                        for q in range(2):
                            nc.vector.scalar_tensor_tensor(
                                out=ot[:, ts(2 * g + q, 2 * NCHUNK)],
                                in0=pys[q][:],
                                scalar=rb[:, i : i + 1],
                                in1=xb_t[:, i, ts(2 * g + q, 2 * NCHUNK)],
                                op0=mybir.AluOpType.mult,
                                op1=mybir.AluOpType.add,
                            )
                    # quarter the last c-tile's writes so the drain tail is
                    # short; halves elsewhere
                    nout = 4 if i == CT - 1 else 2
                    for h in range(nout):
                        nc.sync.dma_start(
                            out_d[s, :, i, ts(h, HW // nout)],
                            ot[:, ts(h, HW // nout)],
                        )

            # software-pipelined emission over the S=2 samples
            load_phase(0)
            mm1_phase(0)
            load_phase(1)
            t_phase(0)
            mm1_phase(1)
            mm2_phase(0)
            t_phase(1)
            mm2_phase(1)

    nc.compile()
    return nc


_PROGRAM_CACHE = {}


def _get_program(S, C, HW, n_cores):
    key = (S, C, HW, n_cores)
    if key not in _PROGRAM_CACHE:
        _PROGRAM_CACHE[key] = build_program(S, C, HW, n_cores)
    return _PROGRAM_CACHE[key]


def make_in_maps(x: np.ndarray, beta: np.ndarray):
    """Host-side prep: shard over batch, swizzle + downcast both layouts."""
    b, c, h, w = x.shape
    hw = h * w
    S = b // N_CORES
    CT = c // P
    NT = hw // P

    xf = np.asarray(x, dtype=np.float32).reshape(b, c, hw)
    # natural, partition-major: [b, P, CT, HW]
    xn = np.ascontiguousarray(xf.reshape(b, CT, P, hw).transpose(0, 2, 1, 3))
    xb = xn.astype(NP_BF16)
    x8 = xn.astype(NP_FP8)
    # transposed: xt[s, p, j, c] = x[s, c, 128j+p] -> [b, P, NT, C]
    xt = np.ascontiguousarray(
        xf.reshape(b, c, NT, P).transpose(0, 3, 2, 1)
    ).astype(NP_FP8)
    beta_bc = np.ascontiguousarray(
        np.broadcast_to(np.asarray(beta, dtype=np.float32).reshape(1, 1), (P, 1))
    )
    return [
        {
            "xb": xb[core * S : (core + 1) * S],
            "x8": x8[core * S : (core + 1) * S],
            "xt": xt[core * S : (core + 1) * S],
            "beta": beta_bc,
        }
        for core in range(N_CORES)
    ]


def kernel(x: np.ndarray, beta: np.ndarray) -> np.ndarray:
    b, c, h, w = x.shape
    assert (b, c, h, w) == (16, 512, 64, 64), f"unexpected shape {x.shape}"
    hw = h * w
    S = b // N_CORES
    CT = c // P

    nc = _get_program(S, c, hw, N_CORES)
    in_maps = make_in_maps(x, beta)
    res = run_bass_kernel_spmd(nc, in_maps, list(range(N_CORES)))

    out = np.empty((b, P, CT, hw), dtype=NP_BF16)
    for core in range(N_CORES):
        out[core * S : (core + 1) * S] = res.results[core]["out"]
    # [b, P, CT, HW] -> [b, C, HW] fp32
    out = out.transpose(0, 2, 1, 3).astype(np.float32).reshape(b, c, hw)
    return out.reshape(b, c, h, w)


# revision 31
# speedup vs baseline: 1.0336x; 1.0135x over previous
"""Trainium2 Bass kernel for nn_CAM (channel-attention module).

Reference computation per sample (b=16 total):
    xf   = x.reshape(c, h*w)               # [512, 4096]
    attn = softmax(xf @ xf.T, axis=-1)     # [512, 512]
    y    = attn @ xf                       # [512, 4096]
    out  = beta * y + x

Sharding: data-parallel over batch b across 8 NeuronCores (2 samples per
core); the scalar beta is replicated (pre-broadcast to [128, 1] host-side).

Mixed-precision layout (tolerance is 2e-2; matmuls in fp8e4 DoubleRow for
2x PE throughput, I/O in bf16/fp8 to cut HBM traffic):
  - host uploads x three ways: natural bf16 [S, 128, 4, 4096]
    (partition-major swizzle) for the epilogue, natural fp8 for
    matmul2's rhs, and pre-transposed fp8 xt[s, p, j, c] = x[s, c, 128j+p]
    for matmul1 (the Gram matrix needs hw on partitions on both operands;
    transposing on the PE would cost ~30us/core of TensorE time, and
    casting on-device measured 4x slower than modeled on gpsimd).
  - matmul1 (G = xf xf^T): 16 DoubleRow MMs per c-tile (K=256 each).
  - softmax: DVE reduce_max(negate) -> ACT Exp(bias=-max) with fused
    accum_out row-sum.  The 1/rowsum * beta normalization is NOT applied
    to P; it is folded into the epilogue as a per-partition scalar.
  - P^T on the PE (16 transpose blocks), PSUM->SBUF copy casts to fp8.
  - matmul2 (y = P @ xf): 2 DoubleRow MMs per [128, 512] output chunk.
  - epilogue: one DVE scalar_tensor_tensor: out = (psum * rb_c) + x_bf16,
    rb_c = beta / rowsum_c, written as bf16 and upcast on host.
  - the two samples' phases are emitted software-pipelined
    (load0, mm1_0, load1, T_0, mm1_1, mm2_0, T_1, mm2_1) so the PE gap
    while sample s's softmax tail completes is filled by sample s+1's
    matmul1.
"""

import numpy as np
import ml_dtypes

import concourse.bass as bass
import concourse.bacc as bacc
import concourse.mybir as mybir
import concourse.tile as tile
from concourse.bass import ts
from concourse.bass_utils import run_bass_kernel_spmd
from concourse.masks import make_identity

N_CORES = 8
P = 128

F32 = mybir.dt.float32
BF16 = mybir.dt.bfloat16
FP8 = mybir.dt.float8e4

NP_BF16 = ml_dtypes.bfloat16
NP_FP8 = ml_dtypes.float8_e4m3

DR = mybir.MatmulPerfMode.DoubleRow
MM1_PERF_MODE = DR


def _mm(nc, out, lhsT, rhs, start, stop, perf_mode=None, ldw=True):
    """nc.tensor.matmul clone with control over the ldweights field.

    When several consecutive matmuls share the same stationary operand,
    walrus still emits one LDWEIGHTS per matmul (no dedupe), and the
    ~213ns weight load serializes with the ~213ns moving stream.  Passing
    ldweights=False on the repeats skips the reload and nearly doubles
    sustained DoubleRow throughput.
    """
    eng = nc.tensor
    keep_dims = {0}
    if perf_mode is not None:
        keep_dims.add(1)
    ifmap_ap = eng.lower_ap(rhs.opt(keep_dims), opt=False)
    weights_ap = eng.lower_ap(
        lhsT.opt(keep_dims), opt=False, for_matmul_weights=True
    )
    out_ap = eng.lower_ap(out)
    return eng.add_instruction(
        mybir.InstMatmult(
            name=eng.bass.get_next_instruction_name(),
            replication_resolution=0,
            replication_shift_amnt=0,
            replication_num_rows=0,
            start_tensor_calc=start,
            stop_tensor_calc=stop,
            ins=[ifmap_ap, weights_ap],
            outs=[out_ap],
            perf_mode=perf_mode,
            is_transpose=None,
            ifmap_quant_offset=None,
            weights_quant_offset=None,
            bass_skip_group_check=None,
            tile_position=(0, 0),
            tile_size=(128, 128),
            ldweights=None if ldw else False,
        )
    )


def build_program(S=2, C=512, HW=4096, n_cores=N_CORES):
    """Build the SPMD Bass program for one core holding S samples."""
    CT = C // P        # c-tiles (partition tiles of the channel dim)
    NT = HW // P       # n-blocks (contraction tiles for matmul1)
    NCHUNK = 512       # free-dim chunk for matmul2 / epilogue (one PSUM bank)
    NCH = HW // NCHUNK
    XTC = 4            # xt arrives in 4 DMA chunks so matmul1 starts early

    nc = bacc.Bacc(
        "TRN2", target_bir_lowering=False, debug=False, num_devices=n_cores
    )
    # natural x, partition-major: xb[s, p, i, n] = x[s, 128*i + p, n]
    xb_in = nc.dram_tensor("xb", [S, P, CT, HW], BF16, kind="ExternalInput").ap()
    x8_in = nc.dram_tensor("x8", [S, P, CT, HW], FP8, kind="ExternalInput").ap()
    # transposed x: xt[s, p, j, c] = x[s, c, 128*j + p]
    xt_in = nc.dram_tensor("xt", [S, P, NT, C], FP8, kind="ExternalInput").ap()
    beta_in = nc.dram_tensor("beta", [P, 1], F32, kind="ExternalInput").ap()
    out_d = nc.dram_tensor("out", [S, P, CT, HW], BF16, kind="ExternalOutput").ap()

    with tile.TileContext(nc) as tc:
        with (
            tc.tile_pool(name="consts", bufs=1) as consts,
            tc.tile_pool(name="xt", bufs=2) as xt_pool,
            tc.tile_pool(name="xb", bufs=2) as xb_pool,
            tc.tile_pool(name="x8", bufs=2) as x8_pool,
            tc.tile_pool(name="pm", bufs=2) as pm_pool,
            tc.tile_pool(name="pt", bufs=2) as pt_pool,
            tc.tile_pool(name="stats", bufs=8) as stats_pool,
            tc.tile_pool(name="sc", bufs=3) as sc_pool,
            tc.tile_pool(name="outsb", bufs=3) as out_pool,
            tc.tile_pool(name="psumA", bufs=2, space="PSUM") as psumA_pool,
            tc.tile_pool(name="psumY", bufs=1, space="PSUM") as psumY_pool,
            tc.tile_pool(name="psumT", bufs=1, space="PSUM") as psumT_pool,
        ):
            beta_bc = consts.tile([P, 1], F32)
            nc.sync.dma_start(beta_bc[:], beta_in)
            ident = consts.tile([P, P], BF16)
            make_identity(nc, ident[:])

            # per-sample state threaded between phases
            st = [dict() for _ in range(S)]

            def load_phase(s):
                xt_t = xt_pool.tile([P, NT, C], FP8, tag="xt")
                for c in range(XTC):
                    nc.sync.dma_start(
                        xt_t[:, ts(c, NT // XTC), :],
                        xt_in[s, :, ts(c, NT // XTC), :],
                    )
                # xb/x8 issue on the ACT HWDGE ring: the SP sequencer's
                # ~650ns per-DMA issue cost would otherwise serialize the fill
                xb_t = xb_pool.tile([P, CT, HW], BF16, tag="xb")
                x8_t = x8_pool.tile([P, CT, HW], FP8, tag="x8")
                for i in range(CT):
                    nc.scalar.dma_start(x8_t[:, i, :], x8_in[s, :, i, :])
                for i in range(CT):
                    nc.scalar.dma_start(xb_t[:, i, :], xb_in[s, :, i, :])
                st[s].update(xt=xt_t, xb=xb_t, x8=x8_t)

            def mm1_phase(s):
                xt_t = st[s]["xt"]
                pm = pm_pool.tile([P, CT, C], BF16, tag="pm")
                rb = stats_pool.tile([P, CT], F32, tag="rb")
                for i in range(CT):
                    pa = psumA_pool.tile([P, C], F32, tag="psumA")
                    for t in range(NT // 2):
                        nc.tensor.matmul(
                            pa[:],
                            lhsT=xt_t[:, 2 * t : 2 * t + 2, ts(i, P)],
                            rhs=xt_t[:, 2 * t : 2 * t + 2, :],
                            start=(t == 0),
                            stop=(t == NT // 2 - 1),
                            perf_mode=MM1_PERF_MODE,
                        )
                    negm = stats_pool.tile([P, 1], F32, tag="negm")
                    nc.vector.reduce_max(
                        negm[:], pa[:], axis=mybir.AxisListType.X, negate=True
                    )
                    ssum = stats_pool.tile([P, 1], F32, tag="ssum")
                    nc.scalar.activation(
                        pm[:, i, :],
                        pa[:],
                        mybir.ActivationFunctionType.Exp,
                        bias=negm[:],
                        scale=1.0,
                        accum_out=ssum[:],
                    )
                    # rb = beta / rowsum; applied in the epilogue
                    rinv = stats_pool.tile([P, 1], F32, tag="rinv")
                    nc.vector.reciprocal(rinv[:], ssum[:])
                    nc.vector.tensor_scalar_mul(
                        rb[:, i : i + 1], rinv[:], beta_bc[:, 0:1]
                    )
                st[s].update(pm=pm, rb=rb)

            def t_phase(s):
                # P^T on PE: PT[p, k, c] = exp(A - m)[c, 128k+p]
                pm = st[s]["pm"]
                PT = pt_pool.tile([P, CT, C], FP8, tag="PT")
                tp = psumT_pool.tile([P, CT, C], BF16, tag="psumT")
                # i-major: the 12 transposes not gated on exp(i=3) run first
                for i in range(CT):
                    for k in range(CT):
                        nc.tensor.transpose(
                            tp[:, k, ts(i, P)], pm[:, i, ts(k, P)], ident[:]
                        )
                for k in range(CT):
                    nc.scalar.copy(PT[:, k, :], tp[:, k, :])
                st[s].update(PT=PT)

            def mm2_phase(s, tiles):
                xb_t, x8_t, PT, rb = (
                    st[s]["xb"], st[s]["x8"], st[s]["PT"], st[s]["rb"]
                )
                # t-outer / n-inner over 4-chunk groups: the stationary weight
                # PT[:, pair, i] is reused across 4 moving streams, amortizing
                # LDWEIGHTS.  Each group uses two 2-bank PSUM tiles; tile 0's
                # epilogue runs as one DVE scalar_tensor_tensor, tile 1's is
                # split ACT scaled-copy + DVE bf16 add (2x DVE rate) so the
                # PSUM drain doesn't gate the matmul stream.
                for i in tiles:
                    ot = out_pool.tile([P, HW], BF16, tag="outsb")
                    for g in range(NCH // 4):
                        pys = [
                            psumY_pool.tile(
                                [P, 2, NCHUNK], F32, tag=f"psumY{q}", name=f"py{q}"
                            )
                            for q in range(2)
                        ]
                        for t in range(CT // 2):
                            for q in range(2):
                                for j in range(2):
                                    n = g * 4 + q * 2 + j
                                    nc.tensor.matmul(
                                        pys[q][:, j, :],
                                        lhsT=PT[:, 2 * t : 2 * t + 2, ts(i, P)],
                                        rhs=x8_t[:, 2 * t : 2 * t + 2, ts(n, NCHUNK)],
                                        start=(t == 0),
                                        stop=(t == CT // 2 - 1),
                                        perf_mode=DR,
                                    )
                        # out = (y * beta/rowsum) + x   over [P, 1024] halves
                        # (DVE only: ACT reads PSUM at half DVE's rate, Pool
                        # at a quarter -- measured, both lose)
                        for q in range(2):
                            nc.vector.scalar_tensor_tensor(
                                out=ot[:, ts(2 * g + q, 2 * NCHUNK)],
                                in0=pys[q][:],
                                scalar=rb[:, i : i + 1],
                                in1=xb_t[:, i, ts(2 * g + q, 2 * NCHUNK)],
                                op0=mybir.AluOpType.mult,
                                op1=mybir.AluOpType.add,
                            )
                    # quarter the last c-tile's writes so the drain tail is
                    # short; halves elsewhere
                    nout = 4 if i == CT - 1 else 2
                    for h in range(nout):
                        nc.sync.dma_start(
                            out_d[s, :, i, ts(h, HW // nout)],
                            ot[:, ts(h, HW // nout)],
                        )

            # software-pipelined emission over the S=2 samples; each sample's
            # matmul2 is split in half around other PE phases so its DVE
            # epilogue backlog drains while the PE is busy elsewhere
            load_phase(0)
            mm1_phase(0)
            load_phase(1)
            t_phase(0)
            mm2_phase(0, [0, 1])
            mm1_phase(1)
            mm2_phase(0, [2, 3])
            t_phase(1)
            mm2_phase(1, [0, 1])
            mm2_phase(1, [2, 3])

    nc.compile()
    return nc


_PROGRAM_CACHE = {}


def _get_program(S, C, HW, n_cores):
    key = (S, C, HW, n_cores)
    if key not in _PROGRAM_CACHE:
        _PROGRAM_CACHE[key] = build_program(S, C, HW, n_cores)
    return _PROGRAM_CACHE[key]


def make_in_maps(x: np.ndarray, beta: np.ndarray):
    """Host-side prep: shard over batch, swizzle + downcast both layouts."""
    b, c, h, w = x.shape
    hw = h * w
    S = b // N_CORES
    CT = c // P
    NT = hw // P

    xf = np.asarray(x, dtype=np.float32).reshape(b, c, hw)
    # natural, partition-major: [b, P, CT, HW]
    xn = np.ascontiguousarray(xf.reshape(b, CT, P, hw).transpose(0, 2, 1, 3))
    xb = xn.astype(NP_BF16)
    x8 = xn.astype(NP_FP8)
    # transposed: xt[s, p, j, c] = x[s, c, 128j+p] -> [b, P, NT, C]
    xt = np.ascontiguousarray(
        xf.reshape(b, c, NT, P).transpose(0, 3, 2, 1)
    ).astype(NP_FP8)
    beta_bc = np.ascontiguousarray(
        np.broadcast_to(np.asarray(beta, dtype=np.float32).reshape(1, 1), (P, 1))
    )
    return [
        {
            "xb": xb[core * S : (core + 1) * S],
            "x8": x8[core * S : (core + 1) * S],
            "xt": xt[core * S : (core + 1) * S],
            "beta": beta_bc,
        }
        for core in range(N_CORES)
    ]


def kernel(x: np.ndarray, beta: np.ndarray) -> np.ndarray:
    b, c, h, w = x.shape
    assert (b, c, h, w) == (16, 512, 64, 64), f"unexpected shape {x.shape}"
    hw = h * w
    S = b // N_CORES
    CT = c // P

    nc = _get_program(S, c, hw, N_CORES)
    in_maps = make_in_maps(x, beta)
    res = run_bass_kernel_spmd(nc, in_maps, list(range(N_CORES)))

    out = np.empty((b, P, CT, hw), dtype=NP_BF16)
    for core in range(N_CORES):
        out[core * S : (core + 1) * S] = res.results[core]["out"]
    # [b, P, CT, HW] -> [b, C, HW] fp32
    out = out.transpose(0, 2, 1, 3).astype(np.float32).reshape(b, c, hw)
    return out.reshape(b, c, h, w)
